# revision 29
# baseline (speedup 1.0000x reference)
"""Trainium2 Bass kernel: Gaussian-splat covariance from (scaling, rotation).

Math (per point n):
  s   = sigmoid(scaling)*(SMAX-SMIN) + SMIN   (SMIN dropped: <=1e-4 abs)
  q   = rotation / ||rotation||;  r,x,y,z = q
  R   = quaternion rotation matrix (3x3)
  L   = R @ diag(s);  C = L @ L^T;  out = upper-tri 6 of C

Implemented with unnormalized quaternions:  RU = n2*R,  G = 1/n2,
t = sigmoid*A*G,  L = RU*diag(t) = R*diag(s).

Layout strategy (the key to DVE throughput): all intermediates are bf16
component PLANES — tile [128, k*F] holds component c in a contiguous
F-long run per partition.  Every vector op streams long dense step-1 bf16
runs -> DVE 2x_1p packed mode; single-src ops get 2x_2p regardless of
stride, which makes the AoS<->plane conversions cheap tensor_scalar
copies.  ScalarE only runs dense-in/dense-out activations (its strided
writes measured ~4.5x slower).

Row/col permutation of R is chosen so that every E/D/partial-sum write is
a uniform-stride plane AP: slot(i,j) = 3*a_i + b_j, a=(2,0,1), b=id.

8-way data parallel over points; per core T tiles of 128*F points.
"""

import numpy as np

import concourse.bass as bass
import concourse.mybir as mybir
from concourse.tile import TileContext

F32 = mybir.dt.float32
BF16 = mybir.dt.bfloat16
ALU = mybir.AluOpType
ACTF = mybir.ActivationFunctionType

SCALE_MIN = 1e-4
SCALE_MAX = 10.0
A_SC = SCALE_MAX - SCALE_MIN

N_CORES = 8
N_TOTAL = 4_000_000

# Per-core tiling: P_CORE = 128 * sum(F_SCHED) points.
F_PTS = 784
T_TILES = 5
F_SCHED = (392, 784, 784, 784, 784, 392)  # small edge tiles cut ramp/tail
P_CORE = 128 * sum(F_SCHED)  # 501760; 8 cores cover 4,014,080 >= 4e6


def _split_sync_waits(nc, nop_max=1):
    """This container's walrus encodes at most 2 sync waits per instruction
    (and none on Drain). Move excess waits onto dedicated NoOps upstream."""
    n = 0
    for bb in nc.main_func.blocks:
        out = []
        for ins in bb.instructions:
            si = ins.sync_info
            waits = list(si.on_wait) if (si is not None and si.on_wait) else []
            is_drain = type(ins).__name__ == "InstDrain"
            limit = 0 if is_drain and len(waits) > 1 else 1
            if len(waits) > limit:
                keep = waits[-limit:] if limit else []
                extra = waits[:-limit] if limit else waits
                for i0 in range(0, len(extra), nop_max):
                    n += 1
                    nop = mybir.InstNoOp(name=f"waitsplit_{n}", ins=[], outs=[])
                    nop.engine = ins.engine
                    nop.sync_info = mybir.SyncInfo(
                        on_wait=extra[i0 : i0 + nop_max], on_update=[]
                    )
                    out.append(nop)
                ins.sync_info = mybir.SyncInfo(
                    on_wait=keep, on_update=list(si.on_update or [])
                )
            out.append(ins)
        bb.instructions[:] = out
    return n


def build_nc(F=F_PTS, T=T_TILES, pool_split=False, split_waits=True):
    """Build the per-core Bass program. Same program on all 8 cores."""
    nc = bass.Bass()
    P = 128
    fsched = F_SCHED
    npts = P * sum(fsched)

    rot_d = nc.declare_dram_parameter("rotation", [npts, 4], F32, isOutput=False)
    scal_d = nc.declare_dram_parameter("scaling", [npts, 3], F32, isOutput=False)
    out_d = nc.declare_dram_parameter("symm", [npts, 6], F32, isOutput=True)

    with TileContext(nc) as tc:
        with (
            tc.tile_pool(name="io", bufs=2) as io,
            tc.tile_pool(name="sc", bufs=1) as sc,
            tc.tile_pool(name="mid", bufs=1) as mid,
        ):
            fstart = 0
            for t, F in enumerate(fsched):
                rows = slice(fstart * P, (fstart + F) * P)
                fstart += F

                ROT = io.tile([P, 4 * F], F32, tag="rot")
                SCAL = io.tile([P, 3 * F], F32, tag="scal")
                OUT = sc.tile([P, 6 * F], F32, tag="out")
                nc.sync.dma_start(
                    ROT[:], rot_d[rows, :].rearrange("(p f) c -> p (f c)", p=P)
                )
                nc.sync.dma_start(
                    SCAL[:], scal_d[rows, :].rearrange("(p f) c -> p (f c)", p=P)
                )

                # cross-engine tiles: double-buffered
                SIGI = sc.tile([P, 3 * F], BF16, tag="sigi")
                SIGP = sc.tile([P, 3 * F], BF16, tag="sigp")
                L = sc.tile([P, 9 * F], BF16, tag="l")
                LSQ = sc.tile([P, 9 * F], BF16, tag="lsq")
                PP = sc.tile([P, 9 * F], BF16, tag="pp")
                # mid tiles
                P2 = mid.tile([P, 4 * F], BF16, tag="p2")
                SQP = mid.tile([P, 4 * F], BF16, tag="sqp")
                HADP = mid.tile([P, 4 * F], BF16, tag="hadp")
                NX = mid.tile([P, F], BF16, tag="nx")
                Y0B = mid.tile([P, F], BF16, tag="y0b")
                YM = mid.tile([P, F], BF16, tag="ym")
                GB = mid.tile([P, F], BF16, tag="gb")
                T3 = mid.tile([P, 3 * F], BF16, tag="t3")
                PRD = mid.tile([P, 6 * F], BF16, tag="prd")
                RU = mid.tile([P, 10 * F], BF16, tag="ru")
                VD = mid.tile([P, 3 * F], BF16, tag="vd")
                VO = mid.tile([P, 3 * F], BF16, tag="vo")
                RES = mid.tile([P, 6 * F], BF16, tag="res")

                ve = nc.vector
                se = nc.scalar
                pe = nc.gpsimd if pool_split else nc.vector

                # views: interleaved tiles walked (f, c); plane tiles as (c, f)
                def inter(tile, k):
                    return tile[:].rearrange("p (f c) -> p f c", c=k)

                def planes_fc(tile, k):
                    # plane tile walked in (f, c) order (matches interleaved)
                    return tile[:].rearrange("p (c f) -> p f c", f=F)

                def pl(tile, k):
                    # plane tile as (p, c, f)
                    return tile[:].rearrange("p (c f) -> p c f", f=F)

                # ---- front-end: ScalarE dense sigmoid; DVE scatter-read
                # copies deposit bf16 planes ----
                se.activation(SIGI[:], SCAL[:], ACTF.Sigmoid)
                # P2 = 2*rot planes [2r 2x 2y 2z]
                ve.tensor_scalar(
                    pl(P2, 4),
                    ROT[:].rearrange("p (f c) -> p c f", c=4),
                    2.0, None, ALU.mult,
                )
                ve.tensor_scalar(
                    pl(SIGP, 3),
                    SIGI[:].rearrange("p (f c) -> p c f", c=3),
                    1.0, None, ALU.mult,
                )
                # SQP = ((2a)/sqrt2)^2 = 2a^2: planes 2rr 2xx 2yy 2zz
                se.activation(SQP[:], P2[:], ACTF.Square, scale=0.7071067811865476)

                q2 = pl(P2, 4)
                sq = pl(SQP, 4)   # planes: 2rr 2xx 2yy 2zz
                had = pl(HADP, 4)  # planes: pm p qm q      (2x scale)
                ru = pl(RU, 10)    # planes 0-8 = 2*Ru; plane 9 = 2*n2
                prd = pl(PRD, 6)  # planes: pxy pyz pxz prz prx pry (2x scale)

                # (p, q) = (rr+xx, yy+zz) -> HADP planes (1,3)
                ve.tensor_tensor(
                    had[:, 1:4:2, :], sq[:, 0:3:2, :], sq[:, 1:4:2, :], ALU.add
                )
                # (pm, qm)*2 -> HADP planes (0,2)
                ve.tensor_tensor(
                    had[:, 0:3:2, :], sq[:, 0:3:2, :], sq[:, 1:4:2, :], ALU.subtract
                )
                # (D1, n2) -> RU planes (1, 9) in one op
                ve.tensor_tensor(
                    ru[:, 1:10:8, :], had[:, 0:2, :], had[:, 2:4, :], ALU.add
                )
                # (D2, D0) -> RU planes (5,6)
                ve.tensor_tensor(
                    ru[:, 5:7, :], had[:, 0:2, :], had[:, 2:4, :], ALU.subtract
                )
                # g ~= 1/(2n2), all bf16: notx = bits(~n2); y0 = notx*c0;
                # ym = (n2*y0 - c1)*y0 = -y1;  GB = -A*ym = A/(2n2).
                n2p = RU[:][:, 9 * F : 10 * F]
                ve.tensor_scalar(
                    NX[:].bitcast(mybir.dt.uint16),
                    n2p.bitcast(mybir.dt.uint16),
                    0xFFFF, None, ALU.bitwise_xor,
                )
                ve.tensor_scalar(Y0B[:], NX[:], -0.23549792, None, ALU.mult)
                ve.tensor_tensor(NX[:], Y0B[:], n2p, ALU.mult)
                ve.scalar_tensor_tensor(
                    YM[:], NX[:], 2.0017324, Y0B[:], ALU.subtract, ALU.mult
                )
                ve.tensor_scalar(GB[:], YM[:], -A_SC, None, ALU.mult)
                # t_j = sigmoid_j * (A/(2*n2))  -> T3 planes (t0,t1,t2)
                t3 = pl(T3, 3)
                ve.tensor_tensor(
                    t3,
                    pl(SIGP, 3),
                    GB[:].unsqueeze(1).broadcast_to((P, 3, F)),
                    ALU.mult,
                )
                # doubled products: (2a)(2b) = 2*(2ab), matching RU scale
                ve.tensor_tensor(prd[:, 0:2, :], q2[:, 1:3, :], q2[:, 2:4, :], ALU.mult)
                ve.tensor_tensor(prd[:, 2:3, :], q2[:, 1:2, :], q2[:, 3:4, :], ALU.mult)
                ve.tensor_tensor(
                    prd[:, 4:6, :],
                    q2[:, 0:1, :].broadcast_to((P, 2, F)),
                    q2[:, 1:3, :], ALU.mult,
                )
                ve.tensor_tensor(prd[:, 3:4, :], q2[:, 0:1, :], q2[:, 3:4, :], ALU.mult)
                # E sums: (E10,E21,E02) -> RU planes (0,4,8)
                ve.tensor_tensor(
                    ru[:, 0:9:4, :], prd[:, 0:3, :], prd[:, 3:6, :], ALU.add
                )
                # E diffs: (E12,E20) -> RU planes (2,3); E01 -> plane 7
                ve.tensor_tensor(
                    ru[:, 2:4, :], prd[:, 1:3, :], prd[:, 4:6, :], ALU.subtract
                )
                ve.tensor_tensor(
                    ru[:, 7:8, :], prd[:, 0:1, :], prd[:, 3:4, :], ALU.subtract
                )

                # ---- L = RU * t (t repeated per row-block) ----
                ru4 = RU[:][:, 0 : 9 * F].rearrange(
                    "p (i j f) -> p i j f", i=3, j=3
                )
                l4 = L[:].rearrange("p (i j f) -> p i j f", i=3, j=3)
                tpat = T3[:].rearrange("p (j f) -> p j f", j=3).unsqueeze(1)
                ve.tensor_tensor(
                    l4, ru4, tpat.broadcast_to((P, 3, 3, F)), ALU.mult
                )

                # ---- LSQ = L^2 on ScalarE (dense) ----
                se.activation(LSQ[:], L[:], ACTF.Square)

                # ---- PP: row-pair products (blocks: 0=row1, 1=row2, 2=row0)
                Lf = L[:]
                PPf = PP[:]
                pe.tensor_tensor(
                    PPf[:, 0 : 3 * F], Lf[:, 6 * F : 9 * F], Lf[:, 0 : 3 * F],
                    ALU.mult,
                )
                pe.tensor_tensor(
                    PPf[:, 3 * F : 6 * F], Lf[:, 6 * F : 9 * F],
                    Lf[:, 3 * F : 6 * F], ALU.mult,
                )
                pe.tensor_tensor(
                    PPf[:, 6 * F : 9 * F], Lf[:, 0 : 3 * F], Lf[:, 3 * F : 6 * F],
                    ALU.mult,
                )

                # ---- reductions over j (planes j0+j1, then +j2) ----
                lsq = pl(LSQ, 9)
                pp = pl(PP, 9)
                vd = pl(VD, 3)
                vo = pl(VO, 3)
                ve.tensor_tensor(vd, lsq[:, 0:9:3, :], lsq[:, 1:9:3, :], ALU.add)
                ve.tensor_tensor(vo, pp[:, 0:9:3, :], pp[:, 1:9:3, :], ALU.add)
                # final sums -> RES planes (C00 C01 C02 C11 C12 C22), dense
                res = pl(RES, 6)
                # diag: blocks (0,1,2) = (C11, C22, C00) -> RES planes (3,5,0)
                ve.tensor_tensor(
                    res[:, 3:6:2, :], vd[:, 0:2, :], lsq[:, 2:6:3, :], ALU.add
                )
                ve.tensor_tensor(
                    res[:, 0:1, :], vd[:, 2:3, :], lsq[:, 8:9, :], ALU.add
                )
                # off-diag: groups (C01, C02, C12) -> RES planes (1,2,4)
                ve.tensor_tensor(
                    res[:, 1:3, :], vo[:, 0:2, :], pp[:, 2:6:3, :], ALU.add
                )
                ve.tensor_tensor(
                    res[:, 4:5, :], vo[:, 2:3, :], pp[:, 8:9, :], ALU.add
                )
                # plane -> interleaved f32 (strided read, dense write)
                ve.tensor_scalar(
                    inter(OUT, 6), planes_fc(RES, 6), 1.0, None, ALU.mult
                )

                # ---- store ----
                nc.sync.dma_start(
                    out_d[rows, :].rearrange("(p f) c -> p (f c)", p=P), OUT[:]
                )
    if split_waits:
        _split_sync_waits(nc)
    return nc


_NC_CACHE = {}


def _get_nc(F, T, pool_split=False):
    key = (F, T, pool_split)
    if key not in _NC_CACHE:
        _NC_CACHE[key] = build_nc(F, T, pool_split)
    return _NC_CACHE[key]


def kernel(scaling: np.ndarray, rotation: np.ndarray) -> np.ndarray:
    from concourse.bass_utils import run_bass_kernel_spmd

    scaling = np.ascontiguousarray(np.asarray(scaling, dtype=np.float32))
    rotation = np.ascontiguousarray(np.asarray(rotation, dtype=np.float32))
    n = scaling.shape[0]

    ntot = N_CORES * P_CORE
    scal_p = np.zeros((ntot, 3), dtype=np.float32)
    rot_p = np.zeros((ntot, 4), dtype=np.float32)
    rot_p[:, 0] = 1.0  # benign quaternion for padding
    scal_p[:n] = scaling
    rot_p[:n] = rotation

    nc = _get_nc(F_PTS, T_TILES)
    in_maps = [
        {
            "scaling": scal_p[i * P_CORE : (i + 1) * P_CORE],
            "rotation": rot_p[i * P_CORE : (i + 1) * P_CORE],
        }
        for i in range(N_CORES)
    ]
    res = run_bass_kernel_spmd(nc, in_maps, list(range(N_CORES)))
    out = np.concatenate([res.results[i]["symm"] for i in range(N_CORES)], axis=0)
    return out[:n]
